# revision 24
# baseline (speedup 1.0000x reference)
"""Trainium2 Bass kernel for CrossStockAttention (sparse top-40 attention).

Strategy (8 NeuronCores, zero inter-core communication):
  - Data-parallel: core = (batch b, query-half). Each core owns 1024 queries of
    one batch and all of that batch's valid keys (compacted, padded to 128).
  - Host-side marshalling: queries permuted valid-first; keys compacted to
    valid-only. Permutation undone on the host after gathering.
  - Ranking trick: cosine top-k per query row is invariant to the positive
    per-row scale 1/|x_q|, so only KEYS are normalized (sim = x_q . x_k/|x_k|).
  - sim runs in float32r (1 cyc/row, ~tf32 precision). Measured f32r matmul
    error ~6e-4 vs the ~1e-2 typical gap between rank-40/41 cosine values, so
    top-40 selection matches fp32.
  - Exact top-40 per valid query via 5 rounds of DVE max8 + match_replace on a
    fp32 work buffer initialized from PSUM (ACT copy) with invalid-query rows
    forced to -1e9 (GPSIMD). The mask is (work == -1e9) computed on GPSIMD:
    removed top-40 slots and invalid-query rows compare equal; padded keys also
    match but have V == 0 and no denominator contribution, so they are inert.
  - mask [q,k] is transposed to maskT [k,q] tile-by-tile with DMA transposes
    (no PE/PSUM/DVE involvement).
  - Attention in transposed score layout S_T[k,q]: softmax denominator arrives
    free as an extra all-ones column appended to V (gated by the valid-key
    indicator). exp() is batched [128,1024] per (head, key-tile).
  - ctx rows are prescaled by 1/denom (ACT ln+exp on the denominator row,
    GPSIMD partition-broadcast, DVE multiply), then the output projection
    accumulates head PAIRS (contraction 128) plus the residual (identity
    matmul of x in f32r) directly in PSUM. LayerNorm mean accumulation is
    fused into the PSUM drain copy.
"""

import math
import numpy as np
import ml_dtypes

import concourse.bass as bass
import concourse.mybir as mybir
from concourse.tile import TileContext
from concourse import bass_utils, bacc

B, N, D, H, TOPK = 4, 2048, 512, 8, 40
DH = D // H
NQ = N // 2          # queries per core
QT = NQ // 128       # query tiles (8)
DC = D // 128        # feature chunks (4)
F32 = mybir.dt.float32
F32R = mybir.dt.float32r
BF16 = mybir.dt.bfloat16
AF = mybir.ActivationFunctionType
ALU = mybir.AluOpType

MASK_KT_DVE = 6      # per head: key-tiles 0..5 masked on DVE, rest on GPSIMD
DEBUG_TAPS = False   # DMA intermediates to DRAM for sim-vs-hw triage


def _chunk3(x, p=128):
    """[A*p, F] -> [p, A, F] (partition-major chunking along dim0)."""
    a = x.shape[0] // p
    return np.ascontiguousarray(x.reshape(a, p, -1).transpose(1, 0, 2))


def _nchunks(total, step=512):
    out = []
    o = 0
    while o < total:
        out.append((o, min(step, total - o)))
        o += step
    return out


def build_nc(KV, VT):
    """Build the single-core program (SPMD across 8 cores, data differs)."""
    KT = KV // 128       # key tiles
    NV = VT * 128        # padded valid-query count
    kv_ch = _nchunks(KV)            # 512-wide chunks (projections)
    sim_ch = _nchunks(KV, 384)      # 384-wide chunks (sim: 1 PSUM bank, f32r)
    v_ch = _nchunks(H * 65)
    NPAIR = H // 2

    nc = bacc.Bacc("TRN2", target_bir_lowering=False, debug=False, num_devices=8)

    xq_t_d = nc.dram_tensor("xq_t", [128, DC, NQ], F32, kind="ExternalInput")
    xq_tb_d = nc.dram_tensor("xq_tb", [128, DC, NQ], BF16, kind="ExternalInput")
    xk_tb_d = nc.dram_tensor("xk_tb", [128, DC, KV], BF16, kind="ExternalInput")
    xk_t_d = nc.dram_tensor("xk_t", [128, DC, KV], F32, kind="ExternalInput")
    wq_t_d = nc.dram_tensor("wq_t", [128, DC, D], BF16, kind="ExternalInput")
    wk_t_d = nc.dram_tensor("wk_t", [128, DC, D], BF16, kind="ExternalInput")
    wv_t_d = nc.dram_tensor("wv_t", [128, DC, H * 65], BF16, kind="ExternalInput")
    wo_p_d = nc.dram_tensor("wo_p", [128, NPAIR, D], BF16, kind="ExternalInput")
    validk_b_d = nc.dram_tensor("validk_b", [1, KV], BF16, kind="ExternalInput")
    extra_rhs_d = nc.dram_tensor("extra_rhs", [1, H * 65], BF16, kind="ExternalInput")
    simbias_b_d = nc.dram_tensor("simbias_b", [1, KV], BF16, kind="ExternalInput")
    onesb_d = nc.dram_tensor("onesb", [1, 128], BF16, kind="ExternalInput")
    selq_d = nc.dram_tensor("selq", [128, VT], F32, kind="ExternalInput")
    negb_d = nc.dram_tensor("negb", [128, VT], F32, kind="ExternalInput")
    lng_d = nc.dram_tensor("lng", [128, D], F32, kind="ExternalInput")
    lnb_d = nc.dram_tensor("lnb", [128, D], F32, kind="ExternalInput")
    bq_col_d = nc.dram_tensor("bq_col", [128, DC], F32, kind="ExternalInput")
    bk_col_d = nc.dram_tensor("bk_col", [128, DC], F32, kind="ExternalInput")
    ident_d = nc.dram_tensor("ident", [128, 128], F32R, kind="ExternalInput")
    ones_col_d = nc.dram_tensor("ones_col", [128, 1], F32, kind="ExternalInput")
    ones_row_d = nc.dram_tensor("ones_row", [1, 128], F32, kind="ExternalInput")
    xq_d = nc.dram_tensor("xq", [128, QT, D], F32R, kind="ExternalInput")
    out_d = nc.dram_tensor("out", [128, QT, D], F32, kind="ExternalOutput")

    dbg = {}
    if DEBUG_TAPS:
        for nm, shape, dt in [
            ("dbg_invn", [1, KV], F32), ("dbg_nrmk", [128, DC, KV], F32),
            ("dbg_work", [128, KV], F32), ("dbg_mask", [128, KV], BF16),
            ("dbg_maskT", [128, KT, NV], BF16),
            ("dbg_expm_pre", [128, KT, NQ], BF16),
            ("dbg_expm", [128, KT, NQ], BF16),
            ("dbg_ctx", [64, NQ], BF16), ("dbg_den", [1, NQ], F32),
            ("dbg_rq", [64, NQ], F32), ("dbg_cpair", [128, NQ], BF16),
            ("dbg_qt", [128, DC, NQ], BF16), ("dbg_kt", [128, DC, KV], BF16),
            ("dbg_vaug", [128, KT, H * 65], BF16),
        ]:
            dbg[nm] = nc.dram_tensor(nm, shape, dt, kind="ExternalOutput")

    with TileContext(nc) as tc:
        with (
            tc.tile_pool(name="consts", bufs=1) as consts,
            tc.tile_pool(name="bigbuf", bufs=1) as bigbuf,
            tc.tile_pool(name="maskp", bufs=2) as maskp,
            tc.tile_pool(name="stream", bufs=2) as stream,
            tc.tile_pool(name="pairs", bufs=NPAIR) as pairs,
            tc.tile_pool(name="psim", bufs=2, space="PSUM") as psim,
            tc.tile_pool(name="pmm", bufs=2, space="PSUM") as pmm,
            tc.tile_pool(name="pcp", bufs=2, space="PSUM") as pcp,
            tc.tile_pool(name="small", bufs=1) as small,
        ):
            # ---- input loads ----
            def load(dram, shape, dtype=F32, pool=consts, tag=None):
                t = pool.tile(shape, dtype, tag=tag or dram.name)
                nc.sync.dma_start(t[:], dram.ap())
                return t

            def load_chunked(dram, shape, dtype, tag):
                t = bigbuf.tile(shape, dtype, tag=tag)
                for c in range(DC):
                    nc.sync.dma_start(t[:, c, :], dram.ap()[:, c, :])
                return t

            # xk_t first: it gates norm -> sim -> topk (the DVE critical path)
            xk_t = load_chunked(xk_t_d, [128, DC, KV], F32, "xkt")
            xq_t = load_chunked(xq_t_d, [128, DC, NQ], F32, "xqt")
            xq_tb = load_chunked(xq_tb_d, [128, DC, NQ], BF16, "xq_tb")
            xk_tb = load_chunked(xk_tb_d, [128, DC, KV], BF16, "xk_tb")
            wq_t = load(wq_t_d, [128, DC, D], BF16)
            wk_t = load(wk_t_d, [128, DC, D], BF16)
            wv_t = load(wv_t_d, [128, DC, H * 65], BF16)
            wo_p = load(wo_p_d, [128, NPAIR, D], BF16)
            validk_b = load(validk_b_d, [1, KV], BF16)
            extra_rhs = load(extra_rhs_d, [1, H * 65], BF16)
            simbias_b = load(simbias_b_d, [1, KV], BF16)
            onesb = load(onesb_d, [1, 128], BF16)
            selq = load(selq_d, [128, VT], F32)
            negb = load(negb_d, [128, VT], F32)
            lng = load(lng_d, [128, D], F32)
            lnb = load(lnb_d, [128, D], F32)
            bq_col = load(bq_col_d, [128, DC], F32)
            bk_col = load(bk_col_d, [128, DC], F32)
            ident_r = load(ident_d, [128, 128], F32R)

            ones_col = load(ones_col_d, [128, 1], F32)
            ones_row = load(ones_row_d, [1, 128], F32)
            eps_col = consts.tile([128, 1], F32)
            nc.vector.memset(eps_col[:], 1.0e-5)
            eps_n = consts.tile([1, 1], F32)
            nc.vector.memset(eps_n[:], 1.0e-20)

            # ---- key normalization: xk_t *= (1/|x_k|) in place ----
            scope_norm = nc.enter_named_scope("p_norm", False)
            invn = maskp.tile([1, KV], F32, tag="work")  # dead before topk
            for (o, n) in sim_ch:
                pn = psim.tile([128, 384], F32, tag="sim")
                for c in range(DC):
                    sq = stream.tile([128, 384], F32, tag="sq")
                    nc.gpsimd.tensor_mul(sq[:, :n], xk_t[:, c, o:o + n],
                                         xk_t[:, c, o:o + n])
                    nc.tensor.matmul(pn[0:1, :n], ones_col[:], sq[:, :n],
                                     start=(c == 0), stop=(c == DC - 1))
                nc.scalar.activation(invn[:, o:o + n], pn[0:1, :n], AF.Ln,
                                     bias=eps_n[:])
            nc.scalar.activation(invn[:], invn[:], AF.Exp, scale=-0.5)
            if DEBUG_TAPS:
                nc.sync.dma_start(dbg["dbg_invn"].ap(), invn[:])
            for (o, n) in sim_ch:
                pb = psim.tile([128, 384], F32, tag="sim")
                nc.tensor.matmul(pb[:, :n], ones_row[:], invn[:, o:o + n],
                                 start=True, stop=True)
                for c in range(DC):
                    nc.vector.tensor_mul(xk_t[:, c, o:o + n], xk_t[:, c, o:o + n],
                                         pb[:, :n])
            if DEBUG_TAPS:
                nc.sync.dma_start(dbg["dbg_nrmk"].ap(), xk_t[:])
            nc.leave_named_scope("p_norm", scope_norm[0], False)

            # ---- sim (f32r) + exact top-40 mask + DMA-transposed maskT ----
            scope_sim = nc.enter_named_scope("p_simtopk", False)
            maskT = bigbuf.tile([128, KT, NV], BF16, tag="xq_tb")  # alias
            scr8 = small.tile([128, 8], F32, tag="scr8")
            for vt in range(VT):
                qs = slice(vt * 128, (vt + 1) * 128)
                work = maskp.tile([128, KV], F32, tag="work")
                for (o, n) in sim_ch:
                    ps = psim.tile([128, 384], F32, tag="sim")
                    nc.tensor.matmul(ps[:, :n], onesb[0:1, :],
                                     simbias_b[:, o:o + n], start=True, stop=False)
                    for c in range(DC):
                        nc.tensor.matmul(ps[:, :n], xq_t[:, c, qs],
                                         xk_t[:, c, o:o + n],
                                         start=False, stop=(c == DC - 1))
                    # fused: work = selq*sim + negb (invalid-query rows -> -1e9,
                    # making their mask all-ones)
                    nc.scalar.activation(work[:, o:o + n], ps[:, :n],
                                         AF.Identity,
                                         scale=selq[:, vt:vt + 1],
                                         bias=negb[:, vt:vt + 1])
                for r in range(TOPK // 8):
                    nc.vector.max(scr8[:], work[:])
                    nc.vector.match_replace(work[:], scr8[:], work[:], -1.0e9)
                if DEBUG_TAPS and vt == 0:
                    nc.sync.dma_start(dbg["dbg_work"].ap(), work[:])
                mask = maskp.tile([128, KV], BF16, tag="mask")
                nc.gpsimd.tensor_scalar(mask[:], work[:], -1.0e9, None,
                                        op0=ALU.is_equal)
                if DEBUG_TAPS and vt == 0:
                    nc.sync.dma_start(dbg["dbg_mask"].ap(), mask[:])
                for kt in range(KT):
                    nc.sync.dma_start_transpose(
                        maskT[:, kt, qs], mask[:, kt * 128:(kt + 1) * 128])
            if DEBUG_TAPS:
                nc.sync.dma_start(dbg["dbg_maskT"].ap(), maskT[:])
            nc.leave_named_scope("p_simtopk", scope_sim[0], False)

            # ---- projections ----
            scope_proj = nc.enter_named_scope("p_proj", False)
            qt_sb = bigbuf.tile([128, DC, NQ], BF16, tag="qt")
            for dot in range(DC):
                ps = pmm.tile([128, NQ], F32, tag="mm")
                for (o, n) in _nchunks(NQ):
                    for c in range(DC):
                        nc.tensor.matmul(
                            ps[:, o:o + n],
                            wq_t[:, c, dot * 128:(dot + 1) * 128],
                            xq_tb[:, c, o:o + n],
                            start=(c == 0), stop=(c == DC - 1))
                nc.scalar.activation(qt_sb[:, dot, :], ps[:],
                                     AF.Identity, bias=bq_col[:, dot:dot + 1])

            kt_sb = bigbuf.tile([128, DC, KV], BF16, tag="kt")
            for dot in range(DC):
                for (o, n) in kv_ch:
                    ps = pmm.tile([128, NQ], F32, tag="mm")
                    for c in range(DC):
                        nc.tensor.matmul(
                            ps[:, :n],
                            wk_t[:, c, dot * 128:(dot + 1) * 128],
                            xk_tb[:, c, o:o + n],
                            start=(c == 0), stop=(c == DC - 1))
                    nc.scalar.activation(kt_sb[:, dot, o:o + n], ps[:, :n],
                                         AF.Identity, bias=bk_col[:, dot:dot + 1])
            if DEBUG_TAPS:
                nc.sync.dma_start(dbg["dbg_qt"].ap(), qt_sb[:])
                nc.sync.dma_start(dbg["dbg_kt"].ap(), kt_sb[:])
            nc.leave_named_scope("p_proj", scope_proj[0], False)

            # ---- attention ----
            scope_att = nc.enter_named_scope("p_attn", False)

            def emit_scores(h):
                hp = (h % 2) * 64
                hc = h // 2
                # 3-deep rotation; every third buffer reuses the dead nrmk slot
                tag = ["expmA", "expmB", "xkt"][h % 3]
                expm = bigbuf.tile([128, KT, NQ], BF16, tag=tag)
                for kt in range(KT):
                    ks = slice(kt * 128, (kt + 1) * 128)
                    ps = pmm.tile([128, NQ], F32, tag="mm")
                    for (o, n) in _nchunks(NQ):
                        nc.tensor.matmul(
                            ps[:, o:o + n],
                            kt_sb[hp:hp + 64, hc, ks],
                            qt_sb[hp:hp + 64, hc, o:o + n],
                            start=True, stop=True)
                    nc.scalar.activation(expm[:, kt, :], ps[:], AF.Exp,
                                         scale=1.0 / math.sqrt(DH))
                if DEBUG_TAPS and h == 0:
                    nc.sync.dma_start(dbg["dbg_expm_pre"].ap(), expm[:])
                return expm

            expm_q = [emit_scores(h) for h in range(3)]

            vaug = bigbuf.tile([128, KT, H * 65], BF16, tag="vaug")
            for kt in range(KT):
                ks = slice(kt * 128, (kt + 1) * 128)
                for (o, n) in v_ch:
                    ps = pmm.tile([128, NQ], F32, tag="mm")
                    nc.tensor.matmul(ps[:, :n], validk_b[0:1, ks],
                                     extra_rhs[:, o:o + n], start=True, stop=False)
                    for c in range(DC):
                        nc.tensor.matmul(
                            ps[:, :n], xk_tb[:, c, ks],
                            wv_t[:, c, o:o + n],
                            start=False, stop=(c == DC - 1))
                    nc.scalar.copy(vaug[:, kt, o:o + n], ps[:, :n])

            if DEBUG_TAPS:
                nc.sync.dma_start(dbg["dbg_vaug"].ap(), vaug[:])
            xq = bigbuf.tile([128, QT, D], F32R, tag="xk_tb")  # alias
            nc.sync.dma_start(xq[:], xq_d.ap())

            def emit_tail(h, expm, cpair):
                for kt in range(KT):
                    eng = nc.vector if kt < MASK_KT_DVE else nc.gpsimd
                    eng.tensor_mul(expm[:, kt, 0:NV], expm[:, kt, 0:NV],
                                   maskT[:, kt, :])
                if DEBUG_TAPS and h == 0:
                    nc.sync.dma_start(dbg["dbg_expm"].ap(), expm[:])
                ctx64 = stream.tile([64, NQ], BF16, tag="ctx")
                denrow = stream.tile([1, NQ], F32, tag="den")
                for (o, n) in _nchunks(NQ):
                    cp = pcp.tile([65, 512], F32, tag="cp")
                    for kt in range(KT):
                        nc.tensor.matmul(cp[:, :n], vaug[:, kt, h * 65:(h + 1) * 65],
                                         expm[:, kt, o:o + n],
                                         start=(kt == 0), stop=(kt == KT - 1))
                    nc.scalar.copy(ctx64[:, o:o + n], cp[0:64, :n])
                    nc.scalar.copy(denrow[:, o:o + n], cp[64:65, :n])
                # 1/denom row via exp(-ln(d)), broadcast to 64 partitions
                rrow = stream.tile([1, NQ], F32, tag="rrow")
                nc.scalar.activation(rrow[:], denrow[:], AF.Ln)
                nc.scalar.activation(rrow[:], rrow[:], AF.Exp, scale=-1.0)
                rq64 = stream.tile([64, NQ], F32, tag="rq64")
                nc.gpsimd.partition_broadcast(rq64[:], rrow[:], channels=64)
                hp = (h % 2) * 64
                nc.vector.tensor_mul(cpair[hp:hp + 64, :], ctx64[:], rq64[:])
                if DEBUG_TAPS and h == 0:
                    nc.sync.dma_start(dbg["dbg_ctx"].ap(), ctx64[:])
                    nc.sync.dma_start(dbg["dbg_den"].ap(), denrow[:])
                    nc.sync.dma_start(dbg["dbg_rq"].ap(), rq64[:])

            cpairs = []
            for h in range(H):
                e = expm_q[h] if h < 3 else emit_scores(h)
                if h % 2 == 0:
                    cpair = pairs.tile([128, NQ], BF16, tag="cpair")
                    cpairs.append(cpair)
                emit_tail(h, e, cpairs[-1])
                if DEBUG_TAPS and h == 1:
                    nc.sync.dma_start(dbg["dbg_cpair"].ap(), cpairs[0][:])
            nc.leave_named_scope("p_attn", scope_att[0], False)

            # ---- output projection + residual + LayerNorm ----
            scope_ln = nc.enter_named_scope("p_ln", False)
            att = bigbuf.tile([128, QT, D], F32, tag="xqt")  # alias
            musum = small.tile([128, QT], F32, tag="musum")
            muneg = small.tile([128, QT], F32, tag="muneg")
            varsum = small.tile([128, QT], F32, tag="varsum")
            rstd = small.tile([128, QT], F32, tag="rstd")
            for qt in range(QT):
                qs = slice(qt * 128, (qt + 1) * 128)
                ps = pmm.tile([128, NQ], F32, tag="mm")
                nc.tensor.matmul(ps[:, 0:D], ident_r[:], xq[:, qt, :],
                                 start=True, stop=False)
                for j in range(NPAIR):
                    nc.tensor.matmul(ps[:, 0:D], cpairs[j][:, qs], wo_p[:, j, :],
                                     start=False, stop=(j == NPAIR - 1))
                nc.vector.tensor_scalar(att[:, qt, :], ps[:, 0:D], 1.0, 0.0,
                                        op0=ALU.mult, op1=ALU.add,
                                        accum_out=musum[:, qt:qt + 1])
            nc.vector.tensor_scalar_mul(muneg[:], musum[:], -1.0 / D)
            for qt in range(QT):
                vtmp = stream.tile([128, D], F32, tag="z")
                nc.vector.scalar_tensor_tensor(
                    vtmp[:], att[:, qt, :], muneg[:, qt:qt + 1], att[:, qt, :],
                    op0=ALU.add, op1=ALU.mult,
                    accum_out=varsum[:, qt:qt + 1])
                nc.vector.scalar_tensor_tensor(
                    att[:, qt, :], att[:, qt, :], muneg[:, qt:qt + 1], lng[:],
                    op0=ALU.add, op1=ALU.mult)
            # rstd = exp(-0.5*ln(var/D + eps)) -- stays in the ln/exp table set
            nc.scalar.activation(rstd[:], varsum[:], AF.Ln,
                                 scale=1.0 / D, bias=eps_col[:])
            nc.scalar.activation(rstd[:], rstd[:], AF.Exp, scale=-0.5)
            for qt in range(QT):
                z = stream.tile([128, D], F32, tag="z")
                nc.vector.scalar_tensor_tensor(
                    z[:], att[:, qt, :], rstd[:, qt:qt + 1], lnb[:],
                    op0=ALU.mult, op1=ALU.add)
                nc.sync.dma_start(out_d.ap()[:, qt, :], z[:])
            nc.leave_named_scope("p_ln", scope_ln[0], False)
    nc.compile()
    return nc


def _prep_core(xb, validb, half, perm_k, KV, VT):
    """Host-side shard prep for one core. Returns (in_map, perm_q, xq)."""
    rows = np.arange(half * NQ, (half + 1) * NQ)
    vr = rows[validb[rows]]
    ir = rows[~validb[rows]]
    perm_q = np.concatenate([vr, ir])
    Vq = len(vr)
    Kv = len(perm_k)

    xq = np.ascontiguousarray(xb[perm_q]).astype(np.float32)          # [NQ, D]
    xk = np.zeros((KV, D), np.float32)
    xk[:Kv] = xb[perm_k]
    validk = np.zeros(KV, np.float32)
    validk[:Kv] = 1.0

    m = {}
    m["xq_t"] = _chunk3(np.ascontiguousarray(xq.T))                   # [128,DC,NQ]
    m["xk_t"] = _chunk3(np.ascontiguousarray(xk.T))                   # [128,DC,KV]
    m["xq_tb"] = m["xq_t"].astype(ml_dtypes.bfloat16)
    m["xk_tb"] = m["xk_t"].astype(ml_dtypes.bfloat16)
    m["validk_b"] = validk[None, :].astype(ml_dtypes.bfloat16)
    m["simbias_b"] = (-1.0e9 * (1.0 - validk))[None, :].astype(ml_dtypes.bfloat16)
    m["onesb"] = np.ones((1, 128), ml_dtypes.bfloat16)
    iq = np.zeros((VT * 128,), np.float32)
    iq[Vq:] = 1.0
    iq = np.ascontiguousarray(iq.reshape(VT, 128).T)                  # [128, VT]
    m["selq"] = 1.0 - iq
    m["negb"] = -1.0e9 * iq
    return m, perm_q, xq


def kernel(stock_features, stock_valid_mask, in_proj_w, in_proj_b,
           out_w, out_b, ln_g, ln_b):
    x = np.asarray(stock_features, np.float32)
    valid = np.asarray(stock_valid_mask).astype(bool)
    W = np.asarray(in_proj_w, np.float32)
    bqkv = np.asarray(in_proj_b, np.float32)
    Wo = np.asarray(out_w, np.float32)
    bo = np.asarray(out_b, np.float32)
    g = np.asarray(ln_g, np.float32)
    be = np.asarray(ln_b, np.float32)

    perm_ks = [np.where(valid[b])[0] for b in range(B)]
    KV = int(math.ceil(max(len(p) for p in perm_ks) / 128.0)) * 128
    Vq_max = max(
        int(valid[b, half * NQ:(half + 1) * NQ].sum())
        for b in range(B) for half in range(2))
    VT = int(math.ceil(Vq_max / 128.0))

    Wq, Wk, Wv = W[:D], W[D:2 * D], W[2 * D:]
    bq, bk, bv = bqkv[:D], bqkv[D:2 * D], bqkv[2 * D:]
    wv_aug = np.zeros((D, H * 65), np.float32)
    rhs_aug = np.zeros((1, H * 65), np.float32)
    for h in range(H):
        wv_aug[:, h * 65:h * 65 + 64] = Wv.T[:, h * 64:(h + 1) * 64]
        rhs_aug[0, h * 65:h * 65 + 64] = bv[h * 64:(h + 1) * 64]
        rhs_aug[0, h * 65 + 64] = 1.0
    shared = {
        "wq_t": _chunk3(np.ascontiguousarray(Wq.T)).astype(ml_dtypes.bfloat16),
        "wk_t": _chunk3(np.ascontiguousarray(Wk.T)).astype(ml_dtypes.bfloat16),
        "wv_t": _chunk3(wv_aug).astype(ml_dtypes.bfloat16),
        "wo_p": np.ascontiguousarray(
            Wo.T.reshape(H // 2, 128, D).transpose(1, 0, 2)
        ).astype(ml_dtypes.bfloat16),
        "extra_rhs": rhs_aug.astype(ml_dtypes.bfloat16),
        "lng": np.ascontiguousarray(np.broadcast_to(g, (128, D))),
        "lnb": np.ascontiguousarray(np.broadcast_to(be, (128, D))),
        "bq_col": np.ascontiguousarray(bq.reshape(DC, 128).T),
        "bk_col": np.ascontiguousarray(bk.reshape(DC, 128).T),
        "ident": np.eye(128, dtype=np.float32),
        "ones_col": np.ones((128, 1), np.float32),
        "ones_row": np.ones((1, 128), np.float32),
    }

    in_maps = []
    perms = []
    for b in range(B):
        for half in range(2):
            m, perm_q, xq = _prep_core(x[b], valid[b], half, perm_ks[b], KV, VT)
            m.update(shared)
            m["xq"] = np.ascontiguousarray(
                (xq + bo[None, :]).reshape(QT, 128, D).transpose(1, 0, 2))
            in_maps.append(m)
            perms.append((b, perm_q))

    nc = build_nc(KV, VT)
    res = bass_utils.run_bass_kernel_spmd(nc, in_maps, core_ids=list(range(8)))

    out = np.zeros((B, N, D), np.float32)
    for core, (b, perm_q) in enumerate(perms):
        o = np.asarray(res.results[core]["out"])      # [128, QT, D]
        out[b, perm_q] = o.transpose(1, 0, 2).reshape(NQ, D)
    return out


# revision 25
# speedup vs baseline: 1.0135x; 1.0135x over previous
"""Trainium2 Bass kernel for CrossStockAttention (sparse top-40 attention).

Strategy (8 NeuronCores, zero inter-core communication):
  - Data-parallel: core = (batch b, query-half). Each core owns 1024 queries of
    one batch and all of that batch's valid keys (compacted, padded to 128).
  - Host-side marshalling: queries permuted valid-first; keys compacted to
    valid-only. Permutation undone on the host after gathering.
  - Ranking trick: cosine top-k per query row is invariant to the positive
    per-row scale 1/|x_q|, so only KEYS are normalized (sim = x_q . x_k/|x_k|).
  - sim runs in float32r (1 cyc/row, ~tf32 precision). Measured f32r matmul
    error ~6e-4 vs the ~1e-2 typical gap between rank-40/41 cosine values, so
    top-40 selection matches fp32.
  - Exact top-40 per valid query via 5 rounds of DVE max8 + match_replace on a
    fp32 work buffer initialized from PSUM (ACT copy) with invalid-query rows
    forced to -1e9 (GPSIMD). The mask is (work == -1e9) computed on GPSIMD:
    removed top-40 slots and invalid-query rows compare equal; padded keys also
    match but have V == 0 and no denominator contribution, so they are inert.
  - mask [q,k] is transposed to maskT [k,q] tile-by-tile with DMA transposes
    (no PE/PSUM/DVE involvement).
  - Attention in transposed score layout S_T[k,q]: softmax denominator arrives
    free as an extra all-ones column appended to V (gated by the valid-key
    indicator). exp() is batched [128,1024] per (head, key-tile).
  - ctx rows are prescaled by 1/denom (ACT ln+exp on the denominator row,
    GPSIMD partition-broadcast, DVE multiply), then the output projection
    accumulates head PAIRS (contraction 128) plus the residual (identity
    matmul of x in f32r) directly in PSUM. LayerNorm mean accumulation is
    fused into the PSUM drain copy.
"""

import math
import numpy as np
import ml_dtypes

import concourse.bass as bass
import concourse.mybir as mybir
from concourse.tile import TileContext
from concourse import bass_utils, bacc

B, N, D, H, TOPK = 4, 2048, 512, 8, 40
DH = D // H
NQ = N // 2          # queries per core
QT = NQ // 128       # query tiles (8)
DC = D // 128        # feature chunks (4)
F32 = mybir.dt.float32
F32R = mybir.dt.float32r
BF16 = mybir.dt.bfloat16
AF = mybir.ActivationFunctionType
ALU = mybir.AluOpType

MASK_KT_DVE = 5      # per head: key-tiles 0..5 masked on DVE, rest on GPSIMD
DEBUG_TAPS = False   # DMA intermediates to DRAM for sim-vs-hw triage


def _chunk3(x, p=128):
    """[A*p, F] -> [p, A, F] (partition-major chunking along dim0)."""
    a = x.shape[0] // p
    return np.ascontiguousarray(x.reshape(a, p, -1).transpose(1, 0, 2))


def _nchunks(total, step=512):
    out = []
    o = 0
    while o < total:
        out.append((o, min(step, total - o)))
        o += step
    return out


def build_nc(KV, VT):
    """Build the single-core program (SPMD across 8 cores, data differs)."""
    KT = KV // 128       # key tiles
    NV = VT * 128        # padded valid-query count
    kv_ch = _nchunks(KV)            # 512-wide chunks (projections)
    sim_ch = _nchunks(KV, 384)      # 384-wide chunks (sim: 1 PSUM bank, f32r)
    v_ch = _nchunks(H * 65)
    NPAIR = H // 2

    nc = bacc.Bacc("TRN2", target_bir_lowering=False, debug=False, num_devices=8)

    xq_t_d = nc.dram_tensor("xq_t", [128, DC, NQ], F32, kind="ExternalInput")
    xq_tb_d = nc.dram_tensor("xq_tb", [128, DC, NQ], BF16, kind="ExternalInput")
    xk_tb_d = nc.dram_tensor("xk_tb", [128, DC, KV], BF16, kind="ExternalInput")
    xk_t_d = nc.dram_tensor("xk_t", [128, DC, KV], F32, kind="ExternalInput")
    wq_t_d = nc.dram_tensor("wq_t", [128, DC, D], BF16, kind="ExternalInput")
    wk_t_d = nc.dram_tensor("wk_t", [128, DC, D], BF16, kind="ExternalInput")
    wv_t_d = nc.dram_tensor("wv_t", [128, DC, H * 65], BF16, kind="ExternalInput")
    wo_p_d = nc.dram_tensor("wo_p", [128, NPAIR, D], BF16, kind="ExternalInput")
    validk_b_d = nc.dram_tensor("validk_b", [1, KV], BF16, kind="ExternalInput")
    extra_rhs_d = nc.dram_tensor("extra_rhs", [1, H * 65], BF16, kind="ExternalInput")
    simbias_b_d = nc.dram_tensor("simbias_b", [1, KV], BF16, kind="ExternalInput")
    onesb_d = nc.dram_tensor("onesb", [1, 128], BF16, kind="ExternalInput")
    selq_d = nc.dram_tensor("selq", [128, VT], F32, kind="ExternalInput")
    negb_d = nc.dram_tensor("negb", [128, VT], F32, kind="ExternalInput")
    lng_d = nc.dram_tensor("lng", [128, D], F32, kind="ExternalInput")
    lnb_d = nc.dram_tensor("lnb", [128, D], F32, kind="ExternalInput")
    bq_col_d = nc.dram_tensor("bq_col", [128, DC], F32, kind="ExternalInput")
    bk_col_d = nc.dram_tensor("bk_col", [128, DC], F32, kind="ExternalInput")
    ident_d = nc.dram_tensor("ident", [128, 128], F32R, kind="ExternalInput")
    ones_col_d = nc.dram_tensor("ones_col", [128, 1], F32, kind="ExternalInput")
    ones_row_d = nc.dram_tensor("ones_row", [1, 128], F32, kind="ExternalInput")
    xq_d = nc.dram_tensor("xq", [128, QT, D], F32R, kind="ExternalInput")
    out_d = nc.dram_tensor("out", [128, QT, D], F32, kind="ExternalOutput")

    dbg = {}
    if DEBUG_TAPS:
        for nm, shape, dt in [
            ("dbg_invn", [1, KV], F32), ("dbg_nrmk", [128, DC, KV], F32),
            ("dbg_work", [128, KV], F32), ("dbg_mask", [128, KV], BF16),
            ("dbg_maskT", [128, KT, NV], BF16),
            ("dbg_expm_pre", [128, KT, NQ], BF16),
            ("dbg_expm", [128, KT, NQ], BF16),
            ("dbg_ctx", [64, NQ], BF16), ("dbg_den", [1, NQ], F32),
            ("dbg_rq", [64, NQ], F32), ("dbg_cpair", [128, NQ], BF16),
            ("dbg_qt", [128, DC, NQ], BF16), ("dbg_kt", [128, DC, KV], BF16),
            ("dbg_vaug", [128, KT, H * 65], BF16),
        ]:
            dbg[nm] = nc.dram_tensor(nm, shape, dt, kind="ExternalOutput")

    with TileContext(nc) as tc:
        with (
            tc.tile_pool(name="consts", bufs=1) as consts,
            tc.tile_pool(name="bigbuf", bufs=1) as bigbuf,
            tc.tile_pool(name="maskp", bufs=2) as maskp,
            tc.tile_pool(name="stream", bufs=2) as stream,
            tc.tile_pool(name="pairs", bufs=NPAIR) as pairs,
            tc.tile_pool(name="psim", bufs=2, space="PSUM") as psim,
            tc.tile_pool(name="pmm", bufs=2, space="PSUM") as pmm,
            tc.tile_pool(name="pcp", bufs=2, space="PSUM") as pcp,
            tc.tile_pool(name="small", bufs=1) as small,
        ):
            # ---- input loads ----
            def load(dram, shape, dtype=F32, pool=consts, tag=None):
                t = pool.tile(shape, dtype, tag=tag or dram.name)
                nc.sync.dma_start(t[:], dram.ap())
                return t

            def load_chunked(dram, shape, dtype, tag):
                t = bigbuf.tile(shape, dtype, tag=tag)
                for c in range(DC):
                    nc.sync.dma_start(t[:, c, :], dram.ap()[:, c, :])
                return t

            # xk_t first: it gates norm -> sim -> topk (the DVE critical path)
            xk_t = load_chunked(xk_t_d, [128, DC, KV], F32, "xkt")
            xq_t = load_chunked(xq_t_d, [128, DC, NQ], F32, "xqt")
            xq_tb = load_chunked(xq_tb_d, [128, DC, NQ], BF16, "xq_tb")
            xk_tb = load_chunked(xk_tb_d, [128, DC, KV], BF16, "xk_tb")
            wq_t = load(wq_t_d, [128, DC, D], BF16)
            wk_t = load(wk_t_d, [128, DC, D], BF16)
            wv_t = load(wv_t_d, [128, DC, H * 65], BF16)
            wo_p = load(wo_p_d, [128, NPAIR, D], BF16)
            validk_b = load(validk_b_d, [1, KV], BF16)
            extra_rhs = load(extra_rhs_d, [1, H * 65], BF16)
            simbias_b = load(simbias_b_d, [1, KV], BF16)
            onesb = load(onesb_d, [1, 128], BF16)
            selq = load(selq_d, [128, VT], F32)
            negb = load(negb_d, [128, VT], F32)
            lng = load(lng_d, [128, D], F32)
            lnb = load(lnb_d, [128, D], F32)
            bq_col = load(bq_col_d, [128, DC], F32)
            bk_col = load(bk_col_d, [128, DC], F32)
            ident_r = load(ident_d, [128, 128], F32R)

            ones_col = load(ones_col_d, [128, 1], F32)
            ones_row = load(ones_row_d, [1, 128], F32)
            eps_col = consts.tile([128, 1], F32)
            nc.vector.memset(eps_col[:], 1.0e-5)
            eps_n = consts.tile([1, 1], F32)
            nc.vector.memset(eps_n[:], 1.0e-20)

            # ---- key normalization: xk_t *= (1/|x_k|) in place ----
            scope_norm = nc.enter_named_scope("p_norm", False)
            invn = maskp.tile([1, KV], F32, tag="work")  # dead before topk
            for (o, n) in sim_ch:
                pn = psim.tile([128, 384], F32, tag="sim")
                for c in range(DC):
                    sq = stream.tile([128, 384], F32, tag="sq")
                    nc.vector.tensor_mul(sq[:, :n], xk_t[:, c, o:o + n],
                                         xk_t[:, c, o:o + n])
                    nc.tensor.matmul(pn[0:1, :n], ones_col[:], sq[:, :n],
                                     start=(c == 0), stop=(c == DC - 1))
                nc.scalar.activation(invn[:, o:o + n], pn[0:1, :n], AF.Ln,
                                     bias=eps_n[:])
            nc.scalar.activation(invn[:], invn[:], AF.Exp, scale=-0.5)
            if DEBUG_TAPS:
                nc.sync.dma_start(dbg["dbg_invn"].ap(), invn[:])
            for (o, n) in sim_ch:
                pb = psim.tile([128, 384], F32, tag="sim")
                nc.tensor.matmul(pb[:, :n], ones_row[:], invn[:, o:o + n],
                                 start=True, stop=True)
                for c in range(DC):
                    nc.vector.tensor_mul(xk_t[:, c, o:o + n], xk_t[:, c, o:o + n],
                                         pb[:, :n])
            if DEBUG_TAPS:
                nc.sync.dma_start(dbg["dbg_nrmk"].ap(), xk_t[:])
            nc.leave_named_scope("p_norm", scope_norm[0], False)

            # ---- sim (f32r) + exact top-40 mask + DMA-transposed maskT ----
            scope_sim = nc.enter_named_scope("p_simtopk", False)
            maskT = bigbuf.tile([128, KT, NV], BF16, tag="xq_tb")  # alias
            scr8 = small.tile([128, 8], F32, tag="scr8")
            for vt in range(VT):
                qs = slice(vt * 128, (vt + 1) * 128)
                work = maskp.tile([128, KV], F32, tag="work")
                for (o, n) in sim_ch:
                    ps = psim.tile([128, 384], F32, tag="sim")
                    nc.tensor.matmul(ps[:, :n], onesb[0:1, :],
                                     simbias_b[:, o:o + n], start=True, stop=False)
                    for c in range(DC):
                        nc.tensor.matmul(ps[:, :n], xq_t[:, c, qs],
                                         xk_t[:, c, o:o + n],
                                         start=False, stop=(c == DC - 1))
                    # fused: work = selq*sim + negb (invalid-query rows -> -1e9,
                    # making their mask all-ones)
                    nc.scalar.activation(work[:, o:o + n], ps[:, :n],
                                         AF.Identity,
                                         scale=selq[:, vt:vt + 1],
                                         bias=negb[:, vt:vt + 1])
                for r in range(TOPK // 8):
                    nc.vector.max(scr8[:], work[:])
                    nc.vector.match_replace(work[:], scr8[:], work[:], -1.0e9)
                if DEBUG_TAPS and vt == 0:
                    nc.sync.dma_start(dbg["dbg_work"].ap(), work[:])
                mask = maskp.tile([128, KV], BF16, tag="mask")
                nc.vector.tensor_scalar(mask[:], work[:], -1.0e9, None,
                                        op0=ALU.is_equal)
                if DEBUG_TAPS and vt == 0:
                    nc.sync.dma_start(dbg["dbg_mask"].ap(), mask[:])
                for kt in range(KT):
                    nc.sync.dma_start_transpose(
                        maskT[:, kt, qs], mask[:, kt * 128:(kt + 1) * 128])
            if DEBUG_TAPS:
                nc.sync.dma_start(dbg["dbg_maskT"].ap(), maskT[:])
            nc.leave_named_scope("p_simtopk", scope_sim[0], False)

            # ---- projections ----
            scope_proj = nc.enter_named_scope("p_proj", False)
            qt_sb = bigbuf.tile([128, DC, NQ], BF16, tag="qt")
            for dot in range(DC):
                ps = pmm.tile([128, NQ], F32, tag="mm")
                for (o, n) in _nchunks(NQ):
                    for c in range(DC):
                        nc.tensor.matmul(
                            ps[:, o:o + n],
                            wq_t[:, c, dot * 128:(dot + 1) * 128],
                            xq_tb[:, c, o:o + n],
                            start=(c == 0), stop=(c == DC - 1))
                nc.scalar.activation(qt_sb[:, dot, :], ps[:],
                                     AF.Identity, bias=bq_col[:, dot:dot + 1])

            kt_sb = bigbuf.tile([128, DC, KV], BF16, tag="kt")
            for dot in range(DC):
                for (o, n) in kv_ch:
                    ps = pmm.tile([128, NQ], F32, tag="mm")
                    for c in range(DC):
                        nc.tensor.matmul(
                            ps[:, :n],
                            wk_t[:, c, dot * 128:(dot + 1) * 128],
                            xk_tb[:, c, o:o + n],
                            start=(c == 0), stop=(c == DC - 1))
                    nc.scalar.activation(kt_sb[:, dot, o:o + n], ps[:, :n],
                                         AF.Identity, bias=bk_col[:, dot:dot + 1])
            if DEBUG_TAPS:
                nc.sync.dma_start(dbg["dbg_qt"].ap(), qt_sb[:])
                nc.sync.dma_start(dbg["dbg_kt"].ap(), kt_sb[:])
            nc.leave_named_scope("p_proj", scope_proj[0], False)

            # ---- attention ----
            scope_att = nc.enter_named_scope("p_attn", False)

            def emit_scores(h):
                hp = (h % 2) * 64
                hc = h // 2
                # 3-deep rotation; every third buffer reuses the dead nrmk slot
                tag = ["expmA", "expmB", "xkt"][h % 3]
                expm = bigbuf.tile([128, KT, NQ], BF16, tag=tag)
                for kt in range(KT):
                    ks = slice(kt * 128, (kt + 1) * 128)
                    ps = pmm.tile([128, NQ], F32, tag="mm")
                    for (o, n) in _nchunks(NQ):
                        nc.tensor.matmul(
                            ps[:, o:o + n],
                            kt_sb[hp:hp + 64, hc, ks],
                            qt_sb[hp:hp + 64, hc, o:o + n],
                            start=True, stop=True)
                    nc.scalar.activation(expm[:, kt, :], ps[:], AF.Exp,
                                         scale=1.0 / math.sqrt(DH))
                if DEBUG_TAPS and h == 0:
                    nc.sync.dma_start(dbg["dbg_expm_pre"].ap(), expm[:])
                return expm

            expm_q = [emit_scores(h) for h in range(3)]

            vaug = bigbuf.tile([128, KT, H * 65], BF16, tag="vaug")
            for kt in range(KT):
                ks = slice(kt * 128, (kt + 1) * 128)
                for (o, n) in v_ch:
                    ps = pmm.tile([128, NQ], F32, tag="mm")
                    nc.tensor.matmul(ps[:, :n], validk_b[0:1, ks],
                                     extra_rhs[:, o:o + n], start=True, stop=False)
                    for c in range(DC):
                        nc.tensor.matmul(
                            ps[:, :n], xk_tb[:, c, ks],
                            wv_t[:, c, o:o + n],
                            start=False, stop=(c == DC - 1))
                    nc.scalar.copy(vaug[:, kt, o:o + n], ps[:, :n])

            if DEBUG_TAPS:
                nc.sync.dma_start(dbg["dbg_vaug"].ap(), vaug[:])
            xq = bigbuf.tile([128, QT, D], F32R, tag="xk_tb")  # alias
            nc.sync.dma_start(xq[:], xq_d.ap())

            def emit_tail(h, expm, cpair):
                for kt in range(KT):
                    eng = nc.vector if kt < MASK_KT_DVE else nc.gpsimd
                    eng.tensor_mul(expm[:, kt, 0:NV], expm[:, kt, 0:NV],
                                   maskT[:, kt, :])
                if DEBUG_TAPS and h == 0:
                    nc.sync.dma_start(dbg["dbg_expm"].ap(), expm[:])
                ctx64 = stream.tile([64, NQ], BF16, tag="ctx")
                denrow = stream.tile([1, NQ], F32, tag="den")
                for (o, n) in _nchunks(NQ):
                    cp = pcp.tile([65, 512], F32, tag="cp")
                    for kt in range(KT):
                        nc.tensor.matmul(cp[:, :n], vaug[:, kt, h * 65:(h + 1) * 65],
                                         expm[:, kt, o:o + n],
                                         start=(kt == 0), stop=(kt == KT - 1))
                    nc.scalar.copy(ctx64[:, o:o + n], cp[0:64, :n])
                    nc.scalar.copy(denrow[:, o:o + n], cp[64:65, :n])
                # 1/denom row on DVE (~18-bit approx), broadcast to 64 rows
                rrow = stream.tile([1, NQ], F32, tag="rrow")
                nc.vector.reciprocal_approx_fast(rrow[:], denrow[:])
                rq64 = stream.tile([64, NQ], F32, tag="rq64")
                nc.gpsimd.partition_broadcast(rq64[:], rrow[:], channels=64)
                hp = (h % 2) * 64
                nc.vector.tensor_mul(cpair[hp:hp + 64, :], ctx64[:], rq64[:])
                if DEBUG_TAPS and h == 0:
                    nc.sync.dma_start(dbg["dbg_ctx"].ap(), ctx64[:])
                    nc.sync.dma_start(dbg["dbg_den"].ap(), denrow[:])
                    nc.sync.dma_start(dbg["dbg_rq"].ap(), rq64[:])

            cpairs = []
            for h in range(H):
                e = expm_q[h] if h < 3 else emit_scores(h)
                if h % 2 == 0:
                    cpair = pairs.tile([128, NQ], BF16, tag="cpair")
                    cpairs.append(cpair)
                emit_tail(h, e, cpairs[-1])
                if DEBUG_TAPS and h == 1:
                    nc.sync.dma_start(dbg["dbg_cpair"].ap(), cpairs[0][:])
            nc.leave_named_scope("p_attn", scope_att[0], False)

            # ---- output projection + residual + LayerNorm ----
            scope_ln = nc.enter_named_scope("p_ln", False)
            att = bigbuf.tile([128, QT, D], F32, tag="xqt")  # alias
            musum = small.tile([128, QT], F32, tag="musum")
            muneg = small.tile([128, QT], F32, tag="muneg")
            varsum = small.tile([128, QT], F32, tag="varsum")
            rstd = small.tile([128, QT], F32, tag="rstd")
            for qt in range(QT):
                qs = slice(qt * 128, (qt + 1) * 128)
                ps = pmm.tile([128, NQ], F32, tag="mm")
                nc.tensor.matmul(ps[:, 0:D], ident_r[:], xq[:, qt, :],
                                 start=True, stop=False)
                for j in range(NPAIR):
                    nc.tensor.matmul(ps[:, 0:D], cpairs[j][:, qs], wo_p[:, j, :],
                                     start=False, stop=(j == NPAIR - 1))
                nc.vector.tensor_scalar(att[:, qt, :], ps[:, 0:D], 1.0, 0.0,
                                        op0=ALU.mult, op1=ALU.add,
                                        accum_out=musum[:, qt:qt + 1])
            nc.vector.tensor_scalar_mul(muneg[:], musum[:], -1.0 / D)
            for qt in range(QT):
                vtmp = stream.tile([128, D], F32, tag="z")
                nc.vector.scalar_tensor_tensor(
                    vtmp[:], att[:, qt, :], muneg[:, qt:qt + 1], att[:, qt, :],
                    op0=ALU.add, op1=ALU.mult,
                    accum_out=varsum[:, qt:qt + 1])
                nc.vector.scalar_tensor_tensor(
                    att[:, qt, :], att[:, qt, :], muneg[:, qt:qt + 1], lng[:],
                    op0=ALU.add, op1=ALU.mult)
            # rstd = exp(-0.5*ln(var/D + eps)) -- stays in the ln/exp table set
            nc.scalar.activation(rstd[:], varsum[:], AF.Ln,
                                 scale=1.0 / D, bias=eps_col[:])
            nc.scalar.activation(rstd[:], rstd[:], AF.Exp, scale=-0.5)
            for qt in range(QT):
                z = stream.tile([128, D], F32, tag="z")
                nc.vector.scalar_tensor_tensor(
                    z[:], att[:, qt, :], rstd[:, qt:qt + 1], lnb[:],
                    op0=ALU.mult, op1=ALU.add)
                nc.sync.dma_start(out_d.ap()[:, qt, :], z[:])
            nc.leave_named_scope("p_ln", scope_ln[0], False)
    nc.compile()
    return nc


def _prep_core(xb, validb, half, perm_k, KV, VT):
    """Host-side shard prep for one core. Returns (in_map, perm_q, xq)."""
    rows = np.arange(half * NQ, (half + 1) * NQ)
    vr = rows[validb[rows]]
    ir = rows[~validb[rows]]
    perm_q = np.concatenate([vr, ir])
    Vq = len(vr)
    Kv = len(perm_k)

    xq = np.ascontiguousarray(xb[perm_q]).astype(np.float32)          # [NQ, D]
    xk = np.zeros((KV, D), np.float32)
    xk[:Kv] = xb[perm_k]
    validk = np.zeros(KV, np.float32)
    validk[:Kv] = 1.0

    m = {}
    m["xq_t"] = _chunk3(np.ascontiguousarray(xq.T))                   # [128,DC,NQ]
    m["xk_t"] = _chunk3(np.ascontiguousarray(xk.T))                   # [128,DC,KV]
    m["xq_tb"] = m["xq_t"].astype(ml_dtypes.bfloat16)
    m["xk_tb"] = m["xk_t"].astype(ml_dtypes.bfloat16)
    m["validk_b"] = validk[None, :].astype(ml_dtypes.bfloat16)
    m["simbias_b"] = (-1.0e9 * (1.0 - validk))[None, :].astype(ml_dtypes.bfloat16)
    m["onesb"] = np.ones((1, 128), ml_dtypes.bfloat16)
    iq = np.zeros((VT * 128,), np.float32)
    iq[Vq:] = 1.0
    iq = np.ascontiguousarray(iq.reshape(VT, 128).T)                  # [128, VT]
    m["selq"] = 1.0 - iq
    m["negb"] = -1.0e9 * iq
    return m, perm_q, xq


def kernel(stock_features, stock_valid_mask, in_proj_w, in_proj_b,
           out_w, out_b, ln_g, ln_b):
    x = np.asarray(stock_features, np.float32)
    valid = np.asarray(stock_valid_mask).astype(bool)
    W = np.asarray(in_proj_w, np.float32)
    bqkv = np.asarray(in_proj_b, np.float32)
    Wo = np.asarray(out_w, np.float32)
    bo = np.asarray(out_b, np.float32)
    g = np.asarray(ln_g, np.float32)
    be = np.asarray(ln_b, np.float32)

    perm_ks = [np.where(valid[b])[0] for b in range(B)]
    KV = int(math.ceil(max(len(p) for p in perm_ks) / 128.0)) * 128
    Vq_max = max(
        int(valid[b, half * NQ:(half + 1) * NQ].sum())
        for b in range(B) for half in range(2))
    VT = int(math.ceil(Vq_max / 128.0))

    Wq, Wk, Wv = W[:D], W[D:2 * D], W[2 * D:]
    bq, bk, bv = bqkv[:D], bqkv[D:2 * D], bqkv[2 * D:]
    wv_aug = np.zeros((D, H * 65), np.float32)
    rhs_aug = np.zeros((1, H * 65), np.float32)
    for h in range(H):
        wv_aug[:, h * 65:h * 65 + 64] = Wv.T[:, h * 64:(h + 1) * 64]
        rhs_aug[0, h * 65:h * 65 + 64] = bv[h * 64:(h + 1) * 64]
        rhs_aug[0, h * 65 + 64] = 1.0
    shared = {
        "wq_t": _chunk3(np.ascontiguousarray(Wq.T)).astype(ml_dtypes.bfloat16),
        "wk_t": _chunk3(np.ascontiguousarray(Wk.T)).astype(ml_dtypes.bfloat16),
        "wv_t": _chunk3(wv_aug).astype(ml_dtypes.bfloat16),
        "wo_p": np.ascontiguousarray(
            Wo.T.reshape(H // 2, 128, D).transpose(1, 0, 2)
        ).astype(ml_dtypes.bfloat16),
        "extra_rhs": rhs_aug.astype(ml_dtypes.bfloat16),
        "lng": np.ascontiguousarray(np.broadcast_to(g, (128, D))),
        "lnb": np.ascontiguousarray(np.broadcast_to(be, (128, D))),
        "bq_col": np.ascontiguousarray(bq.reshape(DC, 128).T),
        "bk_col": np.ascontiguousarray(bk.reshape(DC, 128).T),
        "ident": np.eye(128, dtype=np.float32),
        "ones_col": np.ones((128, 1), np.float32),
        "ones_row": np.ones((1, 128), np.float32),
    }

    in_maps = []
    perms = []
    for b in range(B):
        for half in range(2):
            m, perm_q, xq = _prep_core(x[b], valid[b], half, perm_ks[b], KV, VT)
            m.update(shared)
            m["xq"] = np.ascontiguousarray(
                (xq + bo[None, :]).reshape(QT, 128, D).transpose(1, 0, 2))
            in_maps.append(m)
            perms.append((b, perm_q))

    nc = build_nc(KV, VT)
    res = bass_utils.run_bass_kernel_spmd(nc, in_maps, core_ids=list(range(8)))

    out = np.zeros((B, N, D), np.float32)
    for core, (b, perm_q) in enumerate(perms):
        o = np.asarray(res.results[core]["out"])      # [128, QT, D]
        out[b, perm_q] = o.transpose(1, 0, 2).reshape(NQ, D)
    return out
